# revision 42
# baseline (speedup 1.0000x reference)
"""Trainium2 Bass kernel for graph-contrastive loss (nn_PrePrompt_75496935129282).

Computation (reference):
    self = segment_sum(logits_origin, ori_idx, G)       # [G, D]
    pos  = segment_sum(logits_pos,  pos_idx, G)         # [G, D]
    sim[g, k]  = cos(self[g], pos[k])   (eps-guarded norms)
    res[g] = log(sum_s exp(sim[g, neg_idx[g, s]])) - sim[g, g]
    out = mean(res)

Device strategy (8 NeuronCores, SPMD):
  - GRAPH-partitioned sharding: global graph block b (= g >> 7, 16 blocks of
    128) is owned by core b % 8 as local half j = b // 8. The host routes
    every node to its graph's owner and localizes indices to j*128 + (g%128),
    so each core computes the COMPLETE segment sums for its 256 graphs with
    no reduction collectives at all. Load imbalance is multinomial (~±2%),
    absorbed by padding to a uniform chunk count.
  - Segment sums: one-hot matmuls, 256-node fp8e4m3 DoubleRow chunks
    (default) or 128-node fp16 chunks, accumulating [128, 256] per half in
    PSUM. fp8 one-hots are split: DVE builds the even sub-one-hot on device
    (is_equal vs iota), the host streams the odd one (halves oh DMA bytes).
  - pos phase first: normalize local rows (ACT square/sqrt + DVE approx
    recip), PE-transpose to [d, g], and one fp8 AllGather PER HALF (the
    first triggers mid-phase-1) shares the normalized transposed table;
    every core assembles the full [d, 2048] pos-hat. Collectives have
    ~10-12us fixed latency each plus a variable (~20-45us) bootstrap
    rendezvous, so phase 2 and the gram tiles over AG0's columns overlap
    the gather chain.
  - self rows stay UNnormalized: 1/|s_g| folds into the exp row scale.
    Gram = snT^T @ pn_T (fp16, f32 PSUM), + ln(count) (host-precomputed
    neg_idx multiplicities with the AG column permutation), exp with row
    scale + accumulate -> denominator.
  - Device ships den / sim0_raw / inv-self-norm per local graph; host does
    loss = mean(log(den) - sim0_raw * invna) over all 2048 graphs.
"""

import os
import sys

sys.path.insert(0, "/opt/trn_rl_repo")

import numpy as np

import concourse.bacc as bacc
import concourse.bass as bass  # noqa: F401
import concourse.mybir as mybir
import concourse.tile as tile
from concourse.bass_utils import run_bass_kernel_spmd

# Walrus LDWEIGHTS dedup breaks codegen for this kernel's fp16 matmul mix
# (visitInstLdweights internal error); keep it off unless explicitly set.
if os.environ.get("KERNEL_LDW_OPT", "0") == "1":
    import concourse.bass_utils as _bu

    if not getattr(_bu, "_ldw_opt_patched", False):
        _orig_run_command = _bu.run_command

        def _run_command_ldw(argv, **kw):
            argv = [
                "--enable-ldw-opt=true" if a == "--enable-ldw-opt=false" else a
                for a in argv
            ]
            return _orig_run_command(argv, **kw)

        _bu.run_command = _run_command_ldw
        _bu._ldw_opt_patched = True


def _ensure_ntff_hook():
    """The agent image's antenv lacks axon_hooks; inject it and register
    the ctypes NTFF profiling hook so trace=True works under axon."""
    import types

    import antenv

    if hasattr(antenv, "axon_hooks"):
        return
    mod = types.ModuleType("antenv.axon_hooks")
    mod._hook = None

    def set_axon_ntff_profile_hook(h):
        mod._hook = h

    def get_axon_ntff_profile_hook():
        return mod._hook

    mod.set_axon_ntff_profile_hook = set_axon_ntff_profile_hook
    mod.get_axon_ntff_profile_hook = get_axon_ntff_profile_hook
    sys.modules["antenv.axon_hooks"] = mod
    antenv.axon_hooks = mod
    try:
        from trn_agent_boot.trn_boot import _ntff_profile_via_ctypes

        mod._hook = _ntff_profile_via_ctypes("/opt/axon/libaxon_pjrt.so")
    except Exception as e:  # pragma: no cover
        print(f"ntff hook registration failed: {e}")


F32 = mybir.dt.float32
F16 = mybir.dt.float16
F8 = mybir.dt.float8e4

G = 2048
S = 127
D = 256
NCORES = 8
P = 128
A = 4  # chunks per super-load
JL = 2  # local graph blocks per core (core r owns blocks r and r+8)
GLOC = JL * P

X_DT = os.environ.get("KERNEL_X_DT", "f8")  # f8 | f16


def build_nc(nh: int, mode: str):
    """SPMD Bass program; per-core rows = 2*nh*128 (nh 128-chunks per half)."""
    nchunk = JL * nh
    if mode == "f8":
        ndr = nchunk // 2
        assert ndr % A == 0
        nsup = ndr // A
    else:
        assert nchunk % A == 0
        nsup = nchunk // A

    nc = bacc.Bacc(
        "TRN2",
        target_bir_lowering=False,
        debug=False,
        num_devices=NCORES,
    )
    groups = [list(range(NCORES))]

    # ---- I/O ----
    if mode == "f8":
        xshape = [nsup, P, A, 2, D]
        xdt = F8
    else:
        xshape = [nsup, P, A, D]
        xdt = F16
    xo = nc.dram_tensor("xo", xshape, xdt, kind="ExternalInput").ap()
    xp = nc.dram_tensor("xp", xshape, xdt, kind="ExternalInput").ap()
    if mode == "f8":
        # host-built one-hots for sub i=1; DVE builds sub i=0 on device
        oho = nc.dram_tensor(
            "oho", [nsup, P, A, P], F8, kind="ExternalInput"
        ).ap()
        ohp = nc.dram_tensor(
            "ohp", [nsup, P, A, P], F8, kind="ExternalInput"
        ).ap()
        io_ = nc.dram_tensor("io", [P, ndr], F32, kind="ExternalInput").ap()
        ip_ = nc.dram_tensor("ip", [P, ndr], F32, kind="ExternalInput").ap()
    else:
        oho = ohp = None
        io_ = nc.dram_tensor("io", [P, nchunk], F32, kind="ExternalInput").ap()
        ip_ = nc.dram_tensor("ip", [P, nchunk], F32, kind="ExternalInput").ap()
    lncnt = nc.dram_tensor("lncnt", [JL, P, G], F16, kind="ExternalInput").ap()
    out_d = nc.dram_tensor("out", [P, 6], F32, kind="ExternalOutput").ap()

    # ---- internal DRAM for the AllGathers (one per local half) ----
    ag_in = [
        nc.dram_tensor(f"ag_in{j}", [P, 2, P], F8).ap() for j in range(JL)
    ]  # [d, db, p]
    ag_all = [
        nc.dram_tensor(
            f"ag_all{j}", [NCORES, P, 2, P], F8, addr_space="Shared"
        ).ap()
        for j in range(JL)
    ]

    with tile.TileContext(nc) as tc:
        with (
            tc.tile_pool(name="const", bufs=1) as cpool,
            tc.tile_pool(name="big", bufs=1) as big,
            tc.tile_pool(name="stream", bufs=14) as stream,
            tc.tile_pool(name="oh", bufs=8) as ohpool,
            tc.tile_pool(name="ps_seg", bufs=2, space="PSUM") as pseg,
            tc.tile_pool(name="ps_tr", bufs=2, space="PSUM") as ptr,
            tc.tile_pool(name="ps_gram", bufs=4, space="PSUM") as pgram,
        ):
            # ---- constants ----
            eps_col = cpool.tile([P, 1], F32, tag="eps_col")
            nc.vector.memset(eps_col[:], 1e-16)
            if True:
                iota_i = cpool.tile([P, GLOC], mybir.dt.int32, tag="iota_i")
                nc.gpsimd.iota(
                    iota_i[:], pattern=[[1, GLOC]], base=0, channel_multiplier=0
                )
                iota_f = cpool.tile([P, GLOC], F16, tag="iota_f")
                nc.vector.tensor_copy(iota_f[:], iota_i[:])

            from concourse.masks import make_identity

            ident_f = cpool.tile([P, P], F32, tag="ident_f")
            make_identity(nc, ident_f[:])
            ident = cpool.tile([P, P], F16, tag="ident")
            nc.vector.tensor_copy(ident[:], ident_f[:])

            lnc_sb = big.tile([P, JL, G], F16, tag="lnc")

            # index tables: one DMA each
            itw = ndr if mode == "f8" else nchunk
            it_p = big.tile([P, itw], F32, tag="it_p")
            nc.sync.dma_start(out=it_p[:], in_=ip_)
            it_o = big.tile([P, itw], F32, tag="it_o")
            nc.sync.dma_start(out=it_o[:], in_=io_)

            # persistent tiles
            pl = big.tile([P, JL, D], F16, tag="pl")  # local pos rows (raw)
            sl = big.tile([P, JL, D], F16, tag="sl")  # local self rows (raw)
            pnl = big.tile([P, JL, D], F16, tag="pnl")  # normalized local pos
            pnlT = big.tile([P, 2, JL, P], F8, tag="pnlT")  # [d, db, j, p]
            snT = big.tile([P, 2, JL, P], F8, tag="snT")  # [d, db, j, p]
            pn_T = big.tile([P, 2, G], F8, tag="pn_T")  # [d, db, g-cols]
            n2p = big.tile([P, JL], F32, tag="n2p")
            nap = big.tile([P, JL], F32, tag="nap")
            invp = big.tile([P, JL], F32, tag="invp")
            n2s = big.tile([P, JL], F32, tag="n2s")
            nas = big.tile([P, JL], F32, tag="nas")
            invs = big.tile([P, JL], F32, tag="invs")
            scr = big.tile([P, D], F16, tag="scr")  # square scratch
            escr = big.tile([P, 2, 512], F16, tag="escr")  # exp scratch
            dacc = big.tile([P, JL, 4], F32, tag="dacc")  # den quarters
            s0t = big.tile([P, D], F32, tag="s0t")  # sim0 scratch
            out_sb = big.tile([P, 6], F32, tag="out_sb")

            def emit_norm_inv(src, j, n2, na, inv):
                """inv[:, j] = 1/sqrt(sum_d src[:, j, :]^2 + 1e-16)."""
                nc.scalar.activation(
                    out=scr[:],
                    in_=src[:, j, :],
                    func=mybir.ActivationFunctionType.Square,
                    accum_out=n2[:, j : j + 1],
                )
                nc.scalar.activation(
                    out=na[:, j : j + 1],
                    in_=n2[:, j : j + 1],
                    func=mybir.ActivationFunctionType.Sqrt,
                    bias=eps_col[:],
                )
                nc.vector.reciprocal_approx_fast(
                    out=inv[:, j : j + 1], in_=na[:, j : j + 1]
                )

            def emit_transpose_pair(src, j, dst):
                """dst[d, db, j, p] = src[p, j, db*128+d] for db in 0,1."""
                for db in range(2):
                    tps = ptr.tile([P, P], F16, tag="tr")
                    nc.tensor.transpose(
                        out=tps[:],
                        in_=src[:, j, db * P : (db + 1) * P],
                        identity=ident[:],
                    )
                    nc.vector.tensor_copy(dst[:, db, j, :], tps[:])

            # ============= segment-sum phases =============
            def segment_phase(x_r, oh_r, it_all, dst, tag, hooks):
                """dst[:, j, :] (fp16) = complete local segment sums.

                hooks: {DR-chunk index -> [emit fns]} — fns are emitted at
                that queue position (deps gate execution)."""
                if mode == "f8":
                    ndr_h = nh // 2
                    for s in range(nsup):
                        qx = nc.sync if s % 2 == 0 else nc.scalar
                        qo = nc.scalar if s % 2 == 0 else nc.sync
                        xt = stream.tile([P, A, 2, D], F8, tag="xt")
                        qx.dma_start(out=xt[:], in_=x_r[s])
                        oht = stream.tile([P, 2, A, P], F8, tag="oht")
                        qo.dma_start(out=oht[:, 1], in_=oh_r[s])
                        for a in range(A):
                            c = s * A + a  # DR-chunk index
                            j = c // ndr_h
                            k = c % ndr_h
                            if k == 0:
                                acc = pseg.tile([P, D], F32, tag="acc")
                                segment_phase.acc = acc
                                seg_accs[(tag, j)] = acc
                            acc = segment_phase.acc
                            nc.vector.tensor_scalar(
                                out=oht[:, 0, a, :],
                                in0=iota_f[:, j * P : (j + 1) * P],
                                scalar1=it_all[:, c : c + 1],
                                scalar2=None,
                                op0=mybir.AluOpType.is_equal,
                            )
                            nc.tensor.matmul(
                                out=acc[:],
                                lhsT=oht[:, :, a, :],
                                rhs=xt[:, a],
                                start=(k == 0),
                                stop=(k == ndr_h - 1),
                                perf_mode=mybir.MatmulPerfMode.DoubleRow,
                            )
                            if k == ndr_h - 1:
                                nc.vector.tensor_copy(dst[:, j, :], acc[:])
                            for h in hooks.get(c, []):
                                h()
                else:
                    for s in range(nsup):
                        xt = stream.tile([P, A, D], F16, tag="xt")
                        nc.sync.dma_start(out=xt[:], in_=x_r[s])
                        for a in range(A):
                            c = s * A + a
                            j = c // nh
                            k = c % nh
                            if k == 0:
                                acc = pseg.tile([P, D], F32, tag="acc")
                                segment_phase.acc = acc
                                seg_accs[(tag, j)] = acc
                            acc = segment_phase.acc
                            oh = ohpool.tile([P, P], F16, tag="oh")
                            nc.vector.tensor_scalar(
                                out=oh[:],
                                in0=iota_f[:, j * P : (j + 1) * P],
                                scalar1=it_all[:, c : c + 1],
                                scalar2=None,
                                op0=mybir.AluOpType.is_equal,
                            )
                            nc.tensor.matmul(
                                out=acc[:],
                                lhsT=oh[:],
                                rhs=xt[:, a, :],
                                start=(k == 0),
                                stop=(k == nh - 1),
                            )
                            if k == nh - 1:
                                nc.vector.tensor_copy(dst[:, j, :], acc[:])
                            for h in hooks.get(c, []):
                                h()

            # ---- phase 1 (pos): normalize + transpose per half; AG at end
            seg_accs = {}

            def p_half_done(j):
                def f():
                    emit_norm_inv(pl, j, n2p, nap, invp)
                    nc.vector.tensor_scalar(
                        out=pnl[:, j, :],
                        in0=pl[:, j, :],
                        scalar1=invp[:, j : j + 1],
                        scalar2=None,
                        op0=mybir.AluOpType.mult,
                    )
                    emit_transpose_pair(pnl, j, pnlT)
                    nc.gpsimd.dma_start(
                        out=ag_in[j], in_=pnlT[:, :, j, :]
                    )
                    nc.gpsimd.collective_compute(
                        "AllGather",
                        mybir.AluOpType.bypass,
                        replica_groups=groups,
                        ins=[ag_in[j][:]],
                        outs=[ag_all[j][:]],
                    )

                return f

            ndr_h = (nh // 2 if mode == "f8" else nh)
            hp = ndr_h - 1  # last chunk of half 0
            segment_phase(
                xp, ohp, it_p, pl, "p",
                {min(2 * ndr_h - 2, hp + 20): [p_half_done(0)]},
            )

            # ---- phase 2 (origin): squares per half; ONE sqrt+recip at
            # the end (keeps all sqrts before the tail's exps -> no ACT
            # table ping-pong), then preload the exp table while idle.
            def s_half_done(j):
                def f():
                    nc.scalar.activation(
                        out=scr[:],
                        in_=sl[:, j, :],
                        func=mybir.ActivationFunctionType.Square,
                        accum_out=n2s[:, j : j + 1],
                    )
                    emit_transpose_pair(sl, j, snT)
                    if j == JL - 1:
                        nc.scalar.activation(
                            out=nas[:],
                            in_=n2s[:],
                            func=mybir.ActivationFunctionType.Sqrt,
                            bias=eps_col[:],
                        )
                        nc.vector.reciprocal_approx_fast(
                            out=invs[:], in_=nas[:]
                        )
                        # dummy exp: pull the table load off the tail path
                        nc.scalar.activation(
                            out=escr[:, 0, :1],
                            in_=eps_col[:],
                            func=mybir.ActivationFunctionType.Exp,
                        )

                return f

            segment_phase(
                xo, oho, it_o, sl, "s",
                {16: [p_half_done(1)], min(2 * ndr_h - 2, hp + 31): [s_half_done(0)]},
            )
            # scheduler fence: keep AG-blocked tail matmuls from being
            # hoisted ahead of the remaining segment-sum matmuls
            tc.no_sync_barrier()
            s_half_done(1)()

            # ================= tail =================
            nc.sync.dma_start(
                out=lnc_sb[:], in_=lncnt.rearrange("j p g -> p j g")
            )
            # pn_T assembly: one DMA per (half, d-block), split across the
            # two HW DMA queues so both halves land ~concurrently
            for j in range(JL):
                for db, q in ((0, nc.sync), (1, nc.scalar)):
                    q.dma_start(
                        out=pn_T[:, db, j * 1024 : (j + 1) * 1024],
                        in_=ag_all[j][:, :, db].rearrange("r d p -> d r p"),
                    )

            def emit_gram(j, h):
                """PSUM tile [P, 512] = snT_j^T @ pn_T cols, DR over d=256."""
                gt = pgram.tile([P, 512], F32, tag="gram")
                nc.tensor.matmul(
                    out=gt[:],
                    lhsT=snT[:, :, j, :],
                    rhs=pn_T[:, :, h * 512 : (h + 1) * 512],
                    start=True,
                    stop=True,
                    perf_mode=mybir.MatmulPerfMode.DoubleRow,
                )
                return gt

            def emit_stt_exp(j, h, gt):
                nc.vector.scalar_tensor_tensor(
                    out=gt[:],
                    in0=gt[:],
                    scalar=invs[:, j : j + 1],
                    in1=lnc_sb[:, j, h * 512 : (h + 1) * 512],
                    op0=mybir.AluOpType.mult,
                    op1=mybir.AluOpType.add,
                )
                nc.scalar.activation(
                    out=escr[:, 0, :],
                    in_=gt[:],
                    func=mybir.ActivationFunctionType.Exp,
                    accum_out=dacc[:, j, h : h + 1],
                )

            def emit_den(j):
                nc.vector.tensor_tensor(
                    out=dacc[:, j, 0:1],
                    in0=dacc[:, j, 0:1],
                    in1=dacc[:, j, 1:2],
                    op=mybir.AluOpType.add,
                )
                nc.vector.tensor_tensor(
                    out=dacc[:, j, 2:3],
                    in0=dacc[:, j, 2:3],
                    in1=dacc[:, j, 3:4],
                    op=mybir.AluOpType.add,
                )
                nc.vector.tensor_tensor(
                    out=out_sb[:, j : j + 1],
                    in0=dacc[:, j, 0:1],
                    in1=dacc[:, j, 2:3],
                    op=mybir.AluOpType.add,
                )

            def emit_sim0(j):
                nc.vector.tensor_tensor(
                    out=s0t[:],
                    in0=sl[:, j, :],
                    in1=pnl[:, j, :],
                    op=mybir.AluOpType.mult,
                )
                nc.vector.tensor_reduce(
                    out=out_sb[:, 2 + j : 3 + j],
                    in_=s0t[:],
                    axis=mybir.AxisListType.X,
                    op=mybir.AluOpType.add,
                )

            # AG-independent outputs first (run during the gather waits)
            for j in range(JL):
                emit_sim0(j)
                nc.vector.tensor_copy(
                    out_sb[:, 4 + j : 5 + j], invs[:, j : j + 1]
                )
            # cols [0,1024) come from AG0, [1024,2048) from AG1: run all
            # AG0-dependent tiles first so the tail overlaps AG1
            for h in range(4):
                for j in range(JL):
                    gt = emit_gram(j, h)
                    emit_stt_exp(j, h, gt)
            for j in range(JL):
                emit_den(j)

            nc.scalar.dma_start(out=out_d, in_=out_sb[:])

    nc.compile()
    return nc


def _marshal_shard(x8, x16, lidx, nh, mode):
    """Per-core marshalling: nodes already localized (lidx in [0, 256)).

    f16 -> (x_dev [nsup,P,A,D], it_dev [P,nchunk] f32)
    f8  -> (x_dev [nsup,P,A,2,D], oh_dev [nsup,P,A,2,P]) with host one-hots."""
    nchunk = JL * nh
    cap = nh * P
    rows = nchunk * P
    x = x8 if mode == "f8" else x16
    x_lin = np.zeros((rows, D), x.dtype)
    i_lin = np.full((rows,), -1, np.int64)
    for j in range(JL):
        sel = (lidx >= j * P) & (lidx < (j + 1) * P)
        cnt = int(np.count_nonzero(sel))
        assert cnt <= cap, f"half {j} overflow: {cnt} > {cap}"
        x_lin[j * cap : j * cap + cnt] = x[sel]
        i_lin[j * cap : j * cap + cnt] = lidx[sel]
    if mode == "f8":
        import ml_dtypes

        ndr = nchunk // 2
        nsup = ndr // A
        x_dev = np.ascontiguousarray(
            x_lin.reshape(nsup, A, 2, P, D).transpose(0, 3, 1, 2, 4)
        )
        ic = i_lin.reshape(nchunk, P)
        # host one-hots for the ODD 128-chunks (sub i=1); DVE does the even
        ico = ic[1::2]  # [ndr, P]
        base = ((np.arange(ndr) * 2 + 1) // nh) * P
        oh = (
            ico[:, :, None] == base[:, None, None] + np.arange(P)[None, None, :]
        ).astype(ml_dtypes.float8_e4m3fn)
        oh_dev = np.ascontiguousarray(
            oh.reshape(nsup, A, P, P).transpose(0, 2, 1, 3)
        )
        # index table for the EVEN 128-chunks (sub i=0), f32 for is_equal
        it_dev = np.ascontiguousarray(ic[0::2].T.astype(np.float32))
        return x_dev, (oh_dev, it_dev)
    nsup = nchunk // A
    x_dev = np.ascontiguousarray(
        x_lin.reshape(nsup, A, P, D).transpose(0, 2, 1, 3)
    )
    it_dev = np.ascontiguousarray(
        i_lin.reshape(nchunk, P).T.astype(np.float32)
    )
    return x_dev, it_dev


def _to_f8(x):
    import ml_dtypes

    return x.astype(ml_dtypes.float8_e4m3fn)


def _prep_inputs(logits_origin, logits_pos, ori_idx, pos_idx, neg_idx, mode):
    x16o = np.asarray(logits_origin, dtype=np.float16)
    x16p = np.asarray(logits_pos, dtype=np.float16)
    x8o = _to_f8(np.asarray(logits_origin)) if mode == "f8" else None
    x8p = _to_f8(np.asarray(logits_pos)) if mode == "f8" else None
    oi = np.asarray(ori_idx).astype(np.int64)
    pi = np.asarray(pos_idx).astype(np.int64)
    neg = np.asarray(neg_idx)
    n = x16o.shape[0]
    assert x16o.shape == (n, D) and x16p.shape == (n, D)
    assert neg.shape == (G, S)

    # graph-partitioned routing: block b -> core b%8, local half b//8
    def route(gids):
        block = gids >> 7
        core = block % NCORES
        lidx = (block // NCORES) * P + (gids & (P - 1))
        maxh = 0
        for r in range(NCORES):
            for j in range(JL):
                maxh = max(
                    maxh,
                    int(np.count_nonzero((core == r) & (lidx // P == j))),
                )
        return core, lidx, maxh

    core_o, lio, max_o = route(oi)
    core_p, lip, max_p = route(pi)
    nh = -(-max(max_o, max_p) // P)
    nh = -(-nh // 4) * 4  # multiple of 4: A-alignment in both modes

    # ln(count) table with AG column permutation
    cnt = np.zeros((G, G), dtype=np.float64)
    rows = np.repeat(np.arange(G), S)
    np.add.at(cnt, (rows, neg.ravel().astype(np.int64)), 1.0)
    with np.errstate(divide="ignore"):
        lncnt = np.where(cnt > 0, np.log(cnt), -30000.0).astype(np.float32)
    c = np.arange(G)
    gmap = ((c % 1024) // P + 8 * (c // 1024)) * P + (c % P)

    in_maps = []
    for r in range(NCORES):
        mo = core_o == r
        mp = core_p == r
        xo_dev, aux_o = _marshal_shard(
            None if x8o is None else x8o[mo], x16o[mo], lio[mo], nh, mode
        )
        xp_dev, aux_p = _marshal_shard(
            None if x8p is None else x8p[mp], x16p[mp], lip[mp], nh, mode
        )
        lnc_dev = np.empty((JL, P, G), np.float16)
        for j in range(JL):
            gb = r + 8 * j
            lnc_dev[j] = lncnt[gb * P : (gb + 1) * P][:, gmap]
        m = {"xo": xo_dev, "xp": xp_dev, "lncnt": lnc_dev}
        if mode == "f8":
            m["oho"], m["io"] = aux_o
            m["ohp"], m["ip"] = aux_p
        else:
            m["io"] = aux_o
            m["ip"] = aux_p
        in_maps.append(m)
    return in_maps, nh


def kernel(
    logits_origin,
    logits_pos,
    ori_idx,
    pos_idx,
    neg_idx,
    _trace=False,
    _tmpdir=None,
):
    mode = X_DT
    in_maps, nh = _prep_inputs(
        logits_origin, logits_pos, ori_idx, pos_idx, neg_idx, mode
    )
    if _trace:
        _ensure_ntff_hook()
    nc = build_nc(nh, mode)
    res = run_bass_kernel_spmd(
        nc,
        in_maps,
        core_ids=list(range(NCORES)),
        trace=_trace,
        tmpdir=_tmpdir,
    )
    kernel._last_results = res
    total = 0.0
    for r in range(NCORES):
        o = np.asarray(res.results[r]["out"], dtype=np.float64)
        den = o[:, 0:2]
        s0r = o[:, 2:4]
        inv = o[:, 4:6]
        total += float(np.sum(np.log(den) - s0r * inv))
    return np.asarray(np.float32(total / G))


kernel._last_results = None


def _numpy_emulate(logits_origin, logits_pos, ori_idx, pos_idx, neg_idx,
                   mode=None):
    """Pure-numpy emulation of the device algorithm (input quantization
    only) for fast host-logic + precision validation."""
    mode = mode or X_DT
    if mode == "f8":
        xq_o = _to_f8(np.asarray(logits_origin)).astype(np.float64)
        xq_p = _to_f8(np.asarray(logits_pos)).astype(np.float64)
    else:
        xq_o = np.asarray(logits_origin, np.float16).astype(np.float64)
        xq_p = np.asarray(logits_pos, np.float16).astype(np.float64)
    oi = np.asarray(ori_idx).astype(np.int64)
    pi = np.asarray(pos_idx).astype(np.int64)
    neg = np.asarray(neg_idx)

    self_l = np.zeros((G, D))
    pos_l = np.zeros((G, D))
    np.add.at(self_l, oi, xq_o)
    np.add.at(pos_l, pi, xq_p)
    # fp16 quantization of the table values (PSUM->SBUF cast + AG)
    self_l = self_l.astype(np.float16).astype(np.float64)
    pos_l = pos_l.astype(np.float16).astype(np.float64)
    invs = 1.0 / np.sqrt(np.sum(self_l**2, axis=1) + 1e-16)
    invp = 1.0 / np.sqrt(np.sum(pos_l**2, axis=1) + 1e-16)
    ph = (pos_l * invp[:, None]).astype(np.float16).astype(np.float64)
    gram = self_l @ ph.T
    sim = gram * invs[:, None]
    cnt = np.zeros((G, G))
    rows = np.repeat(np.arange(G), S)
    np.add.at(cnt, (rows, neg.ravel().astype(np.int64)), 1.0)
    den = np.sum(np.exp(sim) * cnt, axis=1)
    s0 = np.sum(self_l * ph, axis=1) * invs
    return float(np.mean(np.log(den) - s0))


if __name__ == "__main__":
    rng = np.random.default_rng(0)
    n = 200000
    inputs = {
        "logits_origin": rng.standard_normal((n, D), dtype=np.float32),
        "logits_pos": rng.standard_normal((n, D), dtype=np.float32),
        "ori_idx": rng.integers(0, G, n, dtype=np.int64),
        "pos_idx": rng.integers(0, G, n, dtype=np.int64),
        "neg_idx": rng.integers(0, G, (G, S), dtype=np.int64),
    }

    def np_ref(logits_origin, logits_pos, ori_idx, pos_idx, neg_idx):
        x = logits_origin.astype(np.float64)
        y = logits_pos.astype(np.float64)
        self_l = np.zeros((G, D))
        pos_l = np.zeros((G, D))
        np.add.at(self_l, ori_idx, x)
        np.add.at(pos_l, pos_idx, y)
        eps = 1e-8
        na = np.maximum(np.linalg.norm(self_l, axis=1), eps)
        nb = np.maximum(np.linalg.norm(pos_l, axis=1), eps)
        sh = self_l / na[:, None]
        ph = pos_l / nb[:, None]
        gram = sh @ ph.T
        sim0 = np.einsum("gd,gd->g", sh, ph)
        den = np.array(
            [np.exp(gram[g, neg_idx[g]]).sum() for g in range(G)]
        )
        res = np.log(den) - sim0
        return res.mean()

    expected = np_ref(**inputs)
    if os.environ.get("SELFTEST", "1") == "1":
        for m in ("f16", "f8"):
            emu = _numpy_emulate(**inputs, mode=m)
            err = abs(emu - expected) / max(abs(expected), 1e-12)
            print(f"emulate[{m}]: expected={expected:.6f} emu={emu:.6f} "
                  f"relerr={err:.3e}")
        # validate the marshalling round-trip: rebuild per-core local sums
        # from the device-layout arrays and compare vs direct segment sums
        in_maps, nh = _prep_inputs(**inputs, mode="f16")
        nchunk = JL * nh
        nsup = nchunk // A
        x64 = inputs["logits_origin"].astype(np.float16).astype(np.float64)
        ref_sums = np.zeros((G, D))
        np.add.at(ref_sums, inputs["ori_idx"], x64)
        worst = 0.0
        tot = 0
        for r, m in enumerate(in_maps):
            x_lin = (
                m["xo"].transpose(0, 2, 1, 3).reshape(nchunk * P, D)
            ).astype(np.float64)
            i_lin = m["io"].T.reshape(nchunk * P).astype(np.int64)
            ok = i_lin >= 0
            tot += int(ok.sum())
            loc = np.zeros((GLOC, D))
            np.add.at(loc, i_lin[ok], x_lin[ok])
            for j in range(JL):
                gb = r + 8 * j
                ref = ref_sums[gb * P : (gb + 1) * P]
                got = loc[j * P : (j + 1) * P]
                worst = max(worst, float(np.abs(ref - got).max()))
        print(f"nh={nh} nodes={tot} (expect {n}) marshal_maxabs={worst:.2e}")
    if os.environ.get("RUN_HW", "0") == "1":
        actual = kernel(**inputs)
        err = abs(actual - expected) / max(abs(expected), 1e-12)
        print(f"hw: expected={expected:.6f} actual={float(actual):.6f} "
              f"relerr={err:.3e}")
